# revision 4
# baseline (speedup 1.0000x reference)
"""Mixtral-style MoE block (T=2048, H=1024, F=2048, E=8, top-2) on 8 trn2
NeuronCores.

Strategy: expert-parallel. Each core holds one expert's weights (w1/w2/w3
shards), computes the router (replicated) + its expert's SwiGLU FFN over all
tokens in fp32r, scales by the renormalized top-2 combine weight for its
expert (zero for tokens not routed to it), and a ReduceScatter(add) combines
the per-expert partial outputs; the host concatenates the 8 output shards.
"""
import numpy as np

try:
    import concourse  # noqa: F401
except ImportError:  # pragma: no cover
    import sys
    sys.path.insert(0, "/opt/trn_rl_repo")

from concourse import mybir, bacc
import concourse.tile as tile
from concourse.masks import make_identity
from concourse.bass_utils import run_bass_kernel_spmd

T, H, F, E, TOP_K = 2048, 1024, 2048, 8, 2
P = 128
NCHUNK = T // P      # 16 token chunks
KH = H // P          # 8 k-tiles over H
KF = F // P          # 16 k-tiles over F
NHALF = 2            # T processed in halves (SBUF capacity)
TH = T // NHALF      # 1024 tokens per half
MH = TH // P         # 8 m-tiles per half
MG = 4               # phase-B m-tiles per PSUM group (MG*2 banks)
F32 = mybir.dt.float32
F32R = mybir.dt.float32r
PSUM = "PSUM"

_NC_CACHE = {}


def _router(nc, tc, small, xt_s, gw_s, esel_s, ident):
    """Replicated router: returns c_e [P, NCHUNK, 1] — this core's expert
    combine weight per token (token t = c*128 + p)."""
    with tc.tile_pool(name="psR", bufs=1, space=PSUM) as psR:
        logits_s = small.tile([E, T], F32)
        for n in range(T // 512):
            ps = psR.tile([E, 512], F32, tag="ps_log")
            for k in range(KH):
                nc.tensor.matmul(ps[:], lhsT=gw_s[:, k, :],
                                 rhs=xt_s[:, k, n * 512:(n + 1) * 512],
                                 start=(k == 0), stop=(k == KH - 1))
            nc.vector.tensor_copy(logits_s[:, n * 512:(n + 1) * 512], ps[:])

        lt_ps = psR.tile([P, NCHUNK * E], F32, tag="ps_tr")
        for c in range(NCHUNK):
            nc.tensor.transpose(out=lt_ps[:, c * E:(c + 1) * E],
                                in_=logits_s[:, c * P:(c + 1) * P],
                                identity=ident[:E, :E])
        lg = small.tile([P, NCHUNK, E], F32)
        nc.vector.tensor_copy(lg[:],
                              lt_ps[:].rearrange("p (c e) -> p c e", e=E))

    bc = [P, NCHUNK, E]
    m1 = small.tile([P, NCHUNK, 1], F32)
    nc.vector.reduce_max(m1[:], lg[:], axis=mybir.AxisListType.X)
    ls = small.tile([P, NCHUNK, E], F32)
    nc.vector.tensor_tensor(ls[:], lg[:], m1[:].to_broadcast(bc),
                            op=mybir.AluOpType.subtract)
    mask1 = small.tile([P, NCHUNK, E], F32)
    nc.vector.tensor_scalar(mask1[:], ls[:], 0.0, None,
                            op0=mybir.AluOpType.is_ge)
    masked = small.tile([P, NCHUNK, E], F32)
    nc.vector.scalar_tensor_tensor(out=masked[:], in0=mask1[:], scalar=-1e30,
                                   in1=ls[:], op0=mybir.AluOpType.mult,
                                   op1=mybir.AluOpType.add)
    m2 = small.tile([P, NCHUNK, 1], F32)
    nc.vector.reduce_max(m2[:], masked[:], axis=mybir.AxisListType.X)
    mask12 = small.tile([P, NCHUNK, E], F32)
    nc.vector.tensor_tensor(mask12[:], ls[:], m2[:].to_broadcast(bc),
                            op=mybir.AluOpType.is_ge)
    ex = small.tile([P, NCHUNK, E], F32)
    nc.scalar.activation(ex[:], ls[:], mybir.ActivationFunctionType.Exp)
    wun = small.tile([P, NCHUNK, E], F32)
    nc.vector.tensor_tensor(wun[:], ex[:], mask12[:], op=mybir.AluOpType.mult)
    den = small.tile([P, NCHUNK, 1], F32)
    nc.vector.reduce_sum(den[:], wun[:], axis=mybir.AxisListType.X)
    rden = small.tile([P, NCHUNK, 1], F32)
    nc.vector.reciprocal(rden[:], den[:])
    cw = small.tile([P, NCHUNK, E], F32)
    nc.vector.tensor_tensor(cw[:], wun[:],
                            esel_s[:].unsqueeze(1).to_broadcast(bc),
                            op=mybir.AluOpType.mult)
    cwn = small.tile([P, NCHUNK, E], F32)
    nc.vector.tensor_tensor(cwn[:], cw[:], rden[:].to_broadcast(bc),
                            op=mybir.AluOpType.mult)
    c_e = small.tile([P, NCHUNK, 1], F32)
    nc.vector.reduce_sum(c_e[:], cwn[:], axis=mybir.AxisListType.X)
    return c_e


def build():
    nc = bacc.Bacc("TRN2", target_bir_lowering=False, debug=False,
                   num_devices=E)
    xt = nc.dram_tensor("xt", [H, T], F32R, kind="ExternalInput")
    gw = nc.dram_tensor("gw", [H, E], F32R, kind="ExternalInput")
    esel = nc.dram_tensor("esel", [P, E], F32, kind="ExternalInput")
    w1 = nc.dram_tensor("w1", [H, F], F32R, kind="ExternalInput")
    w3 = nc.dram_tensor("w3", [H, F], F32R, kind="ExternalInput")
    w2 = nc.dram_tensor("w2", [F, H], F32R, kind="ExternalInput")
    out_shard = nc.dram_tensor("out_shard", [T // E, H], F32,
                               kind="ExternalOutput")

    cc_in = nc.dram_tensor("cc_in", [T, H], F32, kind="Internal")
    cc_out = nc.dram_tensor("cc_out", [T // E, H], F32, kind="Internal")

    with tile.TileContext(nc) as tc:
        with (
            tc.tile_pool(name="big", bufs=1) as big,
            tc.tile_pool(name="small", bufs=1) as small,
            tc.tile_pool(name="wpool", bufs=2) as wpool,
            tc.tile_pool(name="evac", bufs=3) as evac,
        ):
            xt_s = big.tile([P, KH, T], F32R)
            nc.sync.dma_start(out=xt_s[:],
                              in_=xt.ap().rearrange("(k p) t -> p k t", p=P))
            inter = big.tile([P, KF, TH], F32R)  # interT for current half

            gw_s = small.tile([P, KH, E], F32R)
            nc.sync.dma_start(out=gw_s[:],
                              in_=gw.ap().rearrange("(k p) e -> p k e", p=P))
            esel_s = small.tile([P, E], F32)
            nc.sync.dma_start(out=esel_s[:], in_=esel.ap())
            ident = small.tile([P, P], F32)
            make_identity(nc, ident[:])

            c_e = _router(nc, tc, small, xt_s, gw_s, esel_s, ident)

            w1v = w1.ap().rearrange("(k p) f -> p k f", p=P)
            w3v = w3.ap().rearrange("(k p) f -> p k f", p=P)
            w2v = w2.ap().rearrange("(k p) h -> p k h", p=P)
            for th in range(NHALF):
                t0 = th * TH
                # ---- phase A: interT[f, t] = silu(w1.T x) * (w3.T x) ----
                with tc.tile_pool(name=f"psA{th}", bufs=2, space=PSUM) as psA:
                    for f in range(KF):
                        w1f = wpool.tile([P, KH, P], F32R, tag="w1f")
                        nc.sync.dma_start(out=w1f[:],
                                          in_=w1v[:, :, f * P:(f + 1) * P])
                        w3f = wpool.tile([P, KH, P], F32R, tag="w3f")
                        nc.sync.dma_start(out=w3f[:],
                                          in_=w3v[:, :, f * P:(f + 1) * P])
                        for n in range(TH // 512):
                            ts = slice(t0 + n * 512, t0 + (n + 1) * 512)
                            fs = slice(n * 512, (n + 1) * 512)
                            ps1 = psA.tile([P, 512], F32, tag="ps1")
                            for k in range(KH):
                                nc.tensor.matmul(ps1[:], lhsT=w1f[:, k, :],
                                                 rhs=xt_s[:, k, ts],
                                                 start=(k == 0),
                                                 stop=(k == KH - 1))
                            ps3 = psA.tile([P, 512], F32, tag="ps3")
                            for k in range(KH):
                                nc.tensor.matmul(ps3[:], lhsT=w3f[:, k, :],
                                                 rhs=xt_s[:, k, ts],
                                                 start=(k == 0),
                                                 stop=(k == KH - 1))
                            sil = evac.tile([P, 512], F32, tag="sil")
                            nc.scalar.activation(
                                sil[:], ps1[:],
                                mybir.ActivationFunctionType.Silu)
                            nc.vector.tensor_tensor(inter[:, f, fs], sil[:],
                                                    ps3[:],
                                                    op=mybir.AluOpType.mult)
                # ---- phase B: out[t, :] = (interT.T @ w2) * c_e ----
                with tc.tile_pool(name=f"psB{th}", bufs=1, space=PSUM) as psB:
                    for g in range(MH // MG):
                        psbs = [[psB.tile([P, 512], F32, tag=f"psb{m}{n}",
                                          name=f"psb{m}{n}")
                                 for n in range(H // 512)] for m in range(MG)]
                        for k in range(KF):
                            w2k = wpool.tile([P, H], F32R, tag="w2k")
                            nc.sync.dma_start(out=w2k[:], in_=w2v[:, k, :])
                            for m in range(MG):
                                ma = g * MG + m
                                for n in range(H // 512):
                                    nc.tensor.matmul(
                                        psbs[m][n][:],
                                        lhsT=inter[:, k, ma * P:(ma + 1) * P],
                                        rhs=w2k[:, n * 512:(n + 1) * 512],
                                        start=(k == 0), stop=(k == KF - 1))
                        for m in range(MG):
                            ma = g * MG + m
                            for n in range(H // 512):
                                o = evac.tile([P, 512], F32, tag="o")
                                nc.vector.tensor_scalar_mul(
                                    o[:], psbs[m][n][:],
                                    c_e[:, th * MH + ma, :])
                                nc.sync.dma_start(
                                    out=cc_in.ap()[
                                        t0 + ma * P:t0 + (ma + 1) * P,
                                        n * 512:(n + 1) * 512],
                                    in_=o[:])

            nc.gpsimd.collective_compute(
                "ReduceScatter", mybir.AluOpType.add,
                replica_groups=[list(range(E))],
                ins=[cc_in.ap()], outs=[cc_out.ap()])
            nc.sync.dma_start(out=out_shard.ap(), in_=cc_out.ap())
    nc.compile()
    return nc


def kernel(hidden_states, gate_w, w1, w2, w3):
    if "nc" not in _NC_CACHE:
        _NC_CACHE["nc"] = build()
    nc = _NC_CACHE["nc"]

    xt = np.ascontiguousarray(hidden_states.T)
    in_maps = []
    for e in range(E):
        sel = np.zeros((P, E), dtype=np.float32)
        sel[:, e] = 1.0
        in_maps.append({
            "xt": xt,
            "gw": np.ascontiguousarray(gate_w),
            "esel": sel,
            "w1": np.ascontiguousarray(w1[e]),
            "w3": np.ascontiguousarray(w3[e]),
            "w2": np.ascontiguousarray(w2[e]),
        })
    res = run_bass_kernel_spmd(nc, in_maps, core_ids=list(range(E)),
                               trace=False)
    out = np.concatenate([res.results[e]["out_shard"] for e in range(E)],
                         axis=0)
    return out.astype(np.float32)
